# revision 25
# baseline (speedup 1.0000x reference)
"""Trainium2 Bass kernel: 3-layer GraphConv GNN + MLP heads, data-parallel over batch.

Contract: kernel(**inputs) takes the FULL unsharded numpy inputs (same keys as
setup_inputs()) and returns (pi, vf) full-shape numpy arrays.

Strategy (per the data-parallel sharding hint):
  - 8 NeuronCores, 128 batch elements each. Graph structure + weights replicated.
  - The fixed graph's gather/scatter is folded on host into a dense normalized
    adjacency A [256, 256]; aggregation becomes a dense matmul on TensorE.
  - Per-core pipeline alternates two SBUF layouts:
      P: [node (2x128 partitions), (batch, feat) free]
      Q: [(batch%4, feat) = 128 partitions, (quad, node) free]
    using A(H W) = (A H) W so each layer is:
      W-GEMM (Q->P, stationary = activation chunk, moving = blockdiag4(W))
      aggregation (P->Q, stationary = activation chunk, moving = A^T)
      bias+relu fused into the PSUM->SBUF copy (ScalarE activation / VectorE
      tensor_scalar), since Q puts features on partitions.
  - All matmul operands are bf16 (1 cycle/row on TRN2 with no min-free-dim,
    vs fp32r's >=256 requirement): halves the W-GEMM streaming and all
    input DMA bytes. PSUM accumulation stays fp32.
  - PSUM: a unified 6-deep pool of [128, 512] bank tiles shared by all big
    phases (deep enough to cover evacuation latency so the PE never stalls
    on a PSUM slot), plus one small-tile bank and one warm-up bank.
  - Layer-3's node-mean is fused into the PSUM evacuation via accum_out
    (per-quad [128,256] evacs each emit hg[:, g]), so no separate reduce.
  - Inputs arrive in FIVE coalesced DMAs on the SP HWDGE queue, in
    criticality order (each dma_start costs ~625ns of serialized descriptor
    time, and transfers drain FIFO): A^T first, then x in two halves
    (L1 starts after the first), then {W1-selectors, ident}, then cbB =
    everything else (lands during L1). One bf16 [128, 1024] output DMA
    (host upcasts).
  - A short chain of dummy matmuls on a zeroed tile runs during the DMA
    preamble so the PE p-state ramp (0.65 -> 1.2 -> 2.4 GHz) completes
    before the first real matmul.
  - Heads: emb GEMM + PE transposes to a [feat, batch] layout, then pi/vf
    share one double-bank PSUM group of 3 matmuls each (vec part, emb part,
    bias row). The v-transpose runs right after the xv load, before L1.
"""

import sys

import numpy as np

try:
    import concourse  # noqa: F401
except ImportError:  # pragma: no cover - fresh-dir fallback
    sys.path.insert(0, "/opt/trn_rl_repo")

import ml_dtypes

import concourse.bacc as bacc
import concourse.bass as bass
import concourse.mybir as mybir
import concourse.tile as tile
from concourse.bass_utils import run_bass_kernel_spmd

F32 = mybir.dt.float32
BF16 = mybir.dt.bfloat16
RELU = mybir.ActivationFunctionType.Relu
ADD = mybir.AluOpType.add
MAX = mybir.AluOpType.max
BF16NP = ml_dtypes.bfloat16

N_CORES = 8
B = 1024          # total batch
NB = B // N_CORES  # batches per core = 128
N = 256           # nodes
F8 = 8            # padded input feature dim (6 -> 8)
H = 32            # hidden feature dim
VEC = 26
DPI = 512
NQ = NB // 4      # quads per core = 32
NBLK = NB // 16   # 16-batch blocks per core = 8
N_WARM = 8        # p-state warm-up matmuls during the DMA preamble
N_TAIL_WARM = 4   # keep-warm matmuls bridging the L3-evac drain to the heads
STAGGER = True    # staggered For_i sem-reset: overlaps timing-loop iterations
HINTS = (mybir.EngineType.PE, mybir.EngineType.Activation, mybir.EngineType.DVE)

# ---- cbA layout (bf16 cols): critical L1 constants -------------------------
CBA_AT = 0          # A^T chunks: [128, 2*256]
CBA_W1 = 512        # w1sel: [128, 4*128]
CBA_IDENT = 1024    # 128x128 identity
CBA_COLS = 1152

# ---- xv layout (bf16 cols): per-invocation inputs --------------------------
XV_X = 0            # x: [128, (blk, c, 128)]  (blk-major, node-chunk inner)
XV_V = 2048         # v: [128, 32] (cols 26:32 zero)
XV_COLS = 2080

# ---- cbB layout (bf16 cols): remaining constants ---------------------------
CBB_WBD2 = 0
CBB_WBD3 = 128
CBB_WEMB = 256      # blockdiag4(W_emb / N)
CBB_WPIV = 384      # rows 0:26
CBB_WPIE = 896      # rows 0:32
CBB_WVFV = 1408
CBB_WVFE = 1920
CBB_ONES = 2432     # row 0: ones [1, 128]
CBB_BEMB = 2560     # row 0: tile(b_emb, 4) [1, 128]
CBB_BPI = 2688      # row 0: b_pi [1, 512]
CBB_BVF = 3200      # row 0: b_vf [1, 512]
CBB_BIASR = 3712    # [128, 3]: tile(b_l, 4) on partitions, col l
CBB_COLS = 3716


def build_nc(repeat: int = 1, use_for_i: bool = False) -> bacc.Bacc:
    """Build the per-core Bass program (SPMD: all cores run this)."""
    nc = bacc.Bacc("TRN2", target_bir_lowering=False, debug=False)

    # ---- DRAM I/O ----------------------------------------------------------
    cba_d = nc.dram_tensor("cba", [128, CBA_COLS], BF16, kind="ExternalInput").ap()
    xv_d = nc.dram_tensor("xv", [128, XV_COLS], BF16, kind="ExternalInput").ap()
    cbb_d = nc.dram_tensor("cbb", [128, CBB_COLS], BF16, kind="ExternalInput").ap()
    o_d = nc.dram_tensor("o", [NB, 2 * DPI], BF16, kind="ExternalOutput").ap()

    with tile.TileContext(nc) as tc:
        with (
            tc.tile_pool(name="consts", bufs=1) as cp,
            tc.tile_pool(name="acts", bufs=1) as ap_,
            tc.tile_pool(name="xp", bufs=2) as xp_,
            tc.tile_pool(name="outs", bufs=2) as op_,
            tc.tile_pool(name="scratch", bufs=3) as sp_,
            tc.tile_pool(name="psum", bufs=1, space="PSUM") as pp_,
        ):
            # ---- constants (loaded once, outside the repeat loop) ----------
            cba = cp.tile([128, CBA_COLS], BF16, tag="cba")
            nc.sync.dma_start(out=cba[:, 0:512], in_=cba_d[:, 0:512])
            cbb = cp.tile([128, CBB_COLS], BF16, tag="cbb")

            at_sb = [cba[:, CBA_AT + c * N:CBA_AT + (c + 1) * N] for c in range(2)]
            w1sel = [cba[:, CBA_W1 + s * 128:CBA_W1 + (s + 1) * 128] for s in range(4)]
            wbd2 = cbb[:, CBB_WBD2:CBB_WBD2 + 128]
            wbd3 = cbb[:, CBB_WBD3:CBB_WBD3 + 128]
            wembbd = cbb[:, CBB_WEMB:CBB_WEMB + 128]
            ident = cba[:, CBA_IDENT:CBA_IDENT + 128]
            wpiv = cbb[0:VEC, CBB_WPIV:CBB_WPIV + DPI]
            wpie = cbb[0:H, CBB_WPIE:CBB_WPIE + DPI]
            wvfv = cbb[0:VEC, CBB_WVFV:CBB_WVFV + DPI]
            wvfe = cbb[0:H, CBB_WVFE:CBB_WVFE + DPI]
            ones1 = cbb[0:1, CBB_ONES:CBB_ONES + 128]
            bembp = cbb[0:1, CBB_BEMB:CBB_BEMB + 128]
            bpi = cbb[0:1, CBB_BPI:CBB_BPI + DPI]
            bvf = cbb[0:1, CBB_BVF:CBB_BVF + DPI]

            # fp32 copies of the per-partition bias columns (scalar operands
            # for relu_bias); tiny per-iteration DVE op (must follow the cbb
            # DMA in program order).
            biasf = cp.tile([128, 4], F32, tag="biasf")
            biasr = [biasf[:, l:l + 1] for l in range(3)]

            # ---- PE p-state warm-up during the DMA preamble ----------------
            warm = cp.tile([128, 512], BF16, tag="warm")
            nc.gpsimd.memset(warm[:], 0.0)
            wq = pp_.tile([128, 512], F32, tag="warmq", bufs=1, name="warmq")
            for _ in range(N_WARM):
                nc.tensor.matmul(wq[:], warm[:, 0:128], warm[:],
                                 start=True, stop=True)

            def body(load_cbb: bool):
                # ---- load inputs (x split so L1 starts after half) -----
                xv1 = xp_.tile([128, 1024], BF16, tag="xv1")
                nc.sync.dma_start(out=xv1[:], in_=xv_d[:, 0:1024])
                xv2 = xp_.tile([128, XV_COLS - 1024], BF16, tag="xv2")
                nc.sync.dma_start(out=xv2[:], in_=xv_d[:, 1024:XV_COLS])
                if load_cbb:
                    nc.sync.dma_start(out=cba[:, 512:CBA_COLS],
                                      in_=cba_d[:, 512:CBA_COLS])
                    # behind x on the SP HWDGE queue: the big const tensor
                    # is only needed from L2 onward.
                    nc.sync.dma_start(out=cbb[:], in_=cbb_d[:])
                nc.vector.tensor_copy(biasf[:, 0:3],
                                      cbb[:, CBB_BIASR:CBB_BIASR + 3])

                def xblk(blk, c):
                    # stationary [node_in_chunk=128, 128 (b16,f8) cols of blk]
                    t = xv1 if blk < 4 else xv2
                    off = blk * 256 + c * 128 - (0 if blk < 4 else 1024)
                    return t[:, off:off + 128]

                vsb = xv2[:, XV_V - 1024:XV_V - 1024 + VEC]

                # Wait-slot discipline: a self-loading matmul has ONE
                # sync-wait slot; extra waits are split onto standalone
                # Ldweights (legal for bf16). "gate" ldweights absorb known
                # producer waits early; PSUM readers for tile t run on engine
                # t%2 so a recycled slot's previous reader matches the next
                # producer's wait engine.
                def gate(t):
                    nc.tensor.ldweights(t.bitcast(BF16))

                gate(xv1[:, 0:1])
                gate(xv2[:, 0:1])

                def relu_bias(par, dst, src, bias_ap, accum=None):
                    if par % 2:
                        nc.scalar.activation(dst, src, RELU, bias=bias_ap,
                                             accum_out=accum)
                    else:
                        nc.vector.tensor_scalar(dst, src, bias_ap, 0.0, ADD,
                                                MAX, accum_out=accum)

                def plain_copy(par, dst, src):
                    if par % 2:
                        nc.scalar.copy(dst, src)
                    else:
                        nc.vector.tensor_copy(dst, src)

                # ---- v transpose (early: off the critical tail path) ----
                vp = pp_.tile([32, 128], F32, tag="u", bufs=7, name="vp")
                nc.tensor.matmul(vp[0:VEC, :], vsb[:], ident[:],
                                 start=True, stop=True)
                vf_t = sp_.tile([VEC, NB], BF16, tag="vft")
                nc.scalar.copy(vf_t[:], vp[0:VEC, :])

                # ---- L1 aggregation (P -> Q): Z1 = (A X)^T-ish ---------
                # z1 layout: [(b16, f8)=128, (blk, n')]; 4 blks per psum tile.
                z1 = ap_.tile([128, NBLK * N], BF16, tag="z1")
                for p in range(NBLK // 2):
                    q = pp_.tile([128, 512], F32, tag="u", bufs=7, name="q1")
                    for half in range(2):
                        blk = 2 * p + half
                        for c in range(2):
                            nc.tensor.matmul(
                                q[:, half * N:(half + 1) * N],
                                xblk(blk, c),
                                at_sb[c], start=(c == 0), stop=(c == 1))
                    plain_copy(p, z1[:, p * 512:(p + 1) * 512], q[:])

                # ---- L1 W-GEMM (Q -> Q): h1 = relu(Z1 W1 + b1) ---------
                # h1 layout: [(b4, f)=128, (g, n')]; one blk (4 selectors)
                # per psum tile.
                h1 = ap_.tile([128, NQ * N], BF16, tag="h1")
                for blk in range(NBLK):
                    gate(z1[:, blk * N:blk * N + 1])
                    for sp in range(2):
                        q = pp_.tile([128, 512], F32, tag="u", bufs=7, name="wq")
                        for half in range(2):
                            s = 2 * sp + half
                            nc.tensor.matmul(
                                q[:, half * N:(half + 1) * N], w1sel[s],
                                z1[:, blk * N:(blk + 1) * N],
                                start=True, stop=True)
                        g0 = blk * 4 + 2 * sp
                        relu_bias(blk + sp, h1[:, g0 * N:(g0 + 2) * N], q[:],
                                  biasr[0])

                # ---- L2 / L3 -------------------------------------------
                # hg[:, g] = sum_n h3[:, g, n], accumulated by L3's evacs.
                hg = ap_.tile([128, NQ], F32, tag="hg")

                def layer(h_in, wbd, bias_ap, h_out):
                    # W phase: 4 quads per double-bank psum tile, one
                    # contiguous copy into y [128, (g, c, m)].
                    # agg phase: 4 quads per tile; relu+bias evac (split
                    # per-quad with accum_out on the last layer).
                    y = sp_.tile([128, NQ * 2 * 128], BF16, tag="y", bufs=1)
                    for q4 in range(4):
                        gate(h_in[:, q4 * N:q4 * N + 1])
                    for gp in range(NQ // 2):
                        w = pp_.tile([128, 512], F32, tag="u", bufs=7, name="wp")
                        for gi in range(2):
                            g = 2 * gp + gi
                            for c in range(2):
                                nc.tensor.matmul(
                                    w[:, gi * 256 + c * 128:gi * 256 + (c + 1) * 128],
                                    h_in[:, g * N + c * 128:g * N + (c + 1) * 128],
                                    wbd[:, 0:128], start=True, stop=True)
                        plain_copy(gp, y[:, gp * 512:(gp + 1) * 512], w[:])
                    for p in range(NQ // 2):
                        if h_out is None:
                            gate(y[:, p * 512:p * 512 + 1])
                        q = pp_.tile([128, 512], F32, tag="u", bufs=7, name="q2")
                        for half in range(2):
                            g = 2 * p + half
                            for c in range(2):
                                nc.tensor.matmul(
                                    q[:, half * N:(half + 1) * N],
                                    y[:, g * 256 + c * 128:g * 256 + (c + 1) * 128],
                                    at_sb[c], start=(c == 0), stop=(c == 1))
                        if h_out is not None:
                            relu_bias(p, h_out[:, p * 512:(p + 1) * 512],
                                      q[:], bias_ap)
                        else:
                            # last layer: per-quad evac, node-sum into hg.
                            # DVE-heavy: ACT pays +187ns per accumulator read.
                            for half in range(2):
                                g = 2 * p + half
                                relu_bias(0 if g % 8 in (0, 1, 3, 4, 6) else 1,
                                          h3[:, g * N:(g + 1) * N],
                                          q[:, half * N:(half + 1) * N],
                                          bias_ap, accum=hg[:, g:g + 1])

                h2 = ap_.tile([128, NQ * N], BF16, tag="h2")
                layer(h1, wbd2, biasr[1], h2)
                h3 = ap_.tile([128, NQ * N], BF16, tag="h3")
                layer(h2, wbd3, biasr[2], None)

                # keep the PE p-state hot while the last L3 evacs drain
                for _ in range(N_TAIL_WARM):
                    nc.tensor.matmul(wq[:], warm[:, 0:128], warm[:],
                                     start=True, stop=True)

                # ---- emb = (hg/256) @ W_emb + b_emb  (layout [g, (b4,e)]) -
                hgb = sp_.tile([128, NQ], BF16, tag="hgb")
                nc.vector.tensor_copy(hgb[:], hg[:])
                gate(hgb[:, 0:1])
                ep = pp_.tile([32, 128], F32, tag="u", bufs=7, name="ep")
                nc.tensor.matmul(ep[:], hgb[:], wembbd[:], start=True,
                                 stop=False)
                nc.tensor.matmul(ep[:], ones1[:, :NQ], bembp[:], start=False,
                                 stop=True)
                embg = sp_.tile([32, 128], BF16, tag="embg")
                nc.vector.tensor_copy(embg[:], ep[:])

                # ---- transpose to [e, (g, b4)] -------------------------
                embf = sp_.tile([32, NB], BF16, tag="embf")
                tp = pp_.tile([32, 128], F32, tag="u", bufs=7, name="tp")
                for b4 in range(4):
                    nc.tensor.matmul(tp[:, b4 * 32:(b4 + 1) * 32],
                                     embg[:, b4 * 32:(b4 + 1) * 32],
                                     ident[:32, :32], start=True, stop=True)
                # [e, (b4, g)] -> [e, (g, b4)] in one strided copy
                nc.vector.tensor_copy(
                    embf.rearrange("p (g c) -> p c g", c=4),
                    tp.rearrange("p (c g) -> p c g", c=4))

                # ---- heads: one double-bank psum group for pi|vf --------
                osb = op_.tile([NB, 2 * DPI], BF16, tag="osb", name="osb")
                for hi, (wv, we, bb) in enumerate(((wpiv, wpie, bpi),
                                                   (wvfv, wvfe, bvf))):
                    sl = pp_.tile([NB, DPI], F32, tag="u", bufs=7,
                                  name=f"pp{hi}")
                    nc.tensor.matmul(sl[:], vf_t[:], wv, start=True, stop=False)
                    nc.tensor.matmul(sl[:], embf[:], we, start=False, stop=False)
                    nc.tensor.matmul(sl[:], ones1[:], bb, start=False, stop=True)
                    relu_bias(hi + 1, osb[:, hi * DPI:(hi + 1) * DPI], sl[:], 0.0)
                nc.sync.dma_start(out=o_d[:], in_=osb[:])

            # one-time gates for every DMA-loaded matmul operand
            for t in (cba, cbb):
                nc.tensor.ldweights(t[0:1, 0:1].bitcast(BF16))

            if use_for_i and repeat > 1:
                nc.sync.dma_start(out=cba[:, 512:CBA_COLS],
                                  in_=cba_d[:, 512:CBA_COLS])
                nc.sync.dma_start(out=cbb[:], in_=cbb_d[:])
                with tc.For_i(0, repeat, 1, staggered_reset=STAGGER,
                              hint_engines=HINTS):
                    body(load_cbb=False)
            else:
                for i in range(repeat):
                    body(load_cbb=(i == 0))

    nc.compile()
    return nc


# ---------------------------------------------------------------------------
# Host-side packing
# ---------------------------------------------------------------------------

def host_pack(inputs: dict) -> list[dict]:
    gf = np.ascontiguousarray(np.asarray(inputs["graph_feats"], dtype=np.float32))
    vec = np.ascontiguousarray(np.asarray(inputs["vector"], dtype=np.float32))
    src = np.asarray(inputs["src"]).astype(np.int64)
    dst = np.asarray(inputs["dst"]).astype(np.int64)
    W1 = np.asarray(inputs["W1"], dtype=np.float32)
    b1 = np.asarray(inputs["b1"], dtype=np.float32)
    W2 = np.asarray(inputs["W2"], dtype=np.float32)
    b2 = np.asarray(inputs["b2"], dtype=np.float32)
    W3 = np.asarray(inputs["W3"], dtype=np.float32)
    b3 = np.asarray(inputs["b3"], dtype=np.float32)
    W_emb = np.asarray(inputs["W_emb"], dtype=np.float32)
    b_emb = np.asarray(inputs["b_emb"], dtype=np.float32)
    W_pi = np.asarray(inputs["W_pi"], dtype=np.float32)
    b_pi = np.asarray(inputs["b_pi"], dtype=np.float32)
    W_vf = np.asarray(inputs["W_vf"], dtype=np.float32)
    b_vf = np.asarray(inputs["b_vf"], dtype=np.float32)

    # normalized dense adjacency (DGL GraphConv norm='both')
    deg_out = np.bincount(src, minlength=N).astype(np.float32)
    deg_in = np.bincount(dst, minlength=N).astype(np.float32)
    inv_o = np.where(deg_out > 0, deg_out ** -0.5, 0.0).astype(np.float32)
    inv_i = np.where(deg_in > 0, deg_in ** -0.5, 0.0).astype(np.float32)
    norm = inv_o[src] * inv_i[dst]
    A = np.zeros((N, N), dtype=np.float32)        # A[d, s]
    np.add.at(A, (dst, src), norm)
    AT = np.ascontiguousarray(A.T)                # AT[n, n'] = A[n', n]

    # ---- cbA ---------------------------------------------------------------
    W1p = np.zeros((F8, H), dtype=np.float32)
    W1p[:6] = W1
    w1sel = np.zeros((4, 128, 128), dtype=np.float32)
    for s in range(4):
        for b4 in range(4):
            bb = s * 4 + b4
            w1sel[s, bb * F8:(bb + 1) * F8, b4 * H:(b4 + 1) * H] = W1p
    cba = np.zeros((128, CBA_COLS), dtype=np.float32)
    cba[:, CBA_AT:CBA_AT + 2 * N] = AT.reshape(2, 128, N).transpose(1, 0, 2) \
        .reshape(128, 2 * N)
    for s in range(4):
        cba[:, CBA_W1 + s * 128:CBA_W1 + (s + 1) * 128] = w1sel[s]
    cba[:, CBA_IDENT:CBA_IDENT + 128] = np.eye(128, dtype=np.float32)
    cba = cba.astype(BF16NP)

    # ---- cbB ---------------------------------------------------------------
    def blockdiag4(Wm):
        out = np.zeros((128, 128), dtype=np.float32)
        for b4 in range(4):
            out[b4 * H:(b4 + 1) * H, b4 * H:(b4 + 1) * H] = Wm
        return out

    cbb = np.zeros((128, CBB_COLS), dtype=np.float32)
    cbb[:, CBB_WBD2:CBB_WBD2 + 128] = blockdiag4(W2)
    cbb[:, CBB_WBD3:CBB_WBD3 + 128] = blockdiag4(W3)
    cbb[:, CBB_WEMB:CBB_WEMB + 128] = blockdiag4(W_emb / np.float32(N))
    cbb[0:VEC, CBB_WPIV:CBB_WPIV + DPI] = W_pi[:VEC]
    cbb[0:H, CBB_WPIE:CBB_WPIE + DPI] = W_pi[VEC:]
    cbb[0:VEC, CBB_WVFV:CBB_WVFV + DPI] = W_vf[:VEC]
    cbb[0:H, CBB_WVFE:CBB_WVFE + DPI] = W_vf[VEC:]
    cbb[0, CBB_ONES:CBB_ONES + 128] = 1.0
    cbb[0, CBB_BEMB:CBB_BEMB + 128] = np.tile(b_emb, 4)
    cbb[0, CBB_BPI:CBB_BPI + DPI] = b_pi
    cbb[0, CBB_BVF:CBB_BVF + DPI] = b_vf
    for l, b in enumerate((b1, b2, b3)):
        cbb[:, CBB_BIASR + l] = np.tile(b, 4)
    cbb = cbb.astype(BF16NP)

    # ---- per-core xv -------------------------------------------------------
    gfp = np.zeros((B, N, F8), dtype=np.float32)
    gfp[:, :, :6] = gf

    in_maps = []
    for core in range(N_CORES):
        gfc = gfp[core * NB:(core + 1) * NB]                  # [128, 256, 8]
        x = np.ascontiguousarray(gfc.transpose(1, 0, 2)).reshape(N, NB * F8)
        xr = x.reshape(2, 128, NBLK, 128)       # [c, node_in_chunk, blk, col]
        xv = np.zeros((128, XV_COLS), dtype=np.float32)
        xv[:, XV_X:XV_X + 2048] = xr.transpose(1, 2, 0, 3).reshape(128, 2048)
        xv[:, XV_V:XV_V + VEC] = vec[core * NB:(core + 1) * NB]
        in_maps.append({
            "cba": cba, "xv": xv.astype(BF16NP), "cbb": cbb,
        })
    return in_maps


_NC_CACHE: dict = {}


def kernel(**inputs):
    key = (1, False)
    if key not in _NC_CACHE:
        _NC_CACHE[key] = build_nc(*key)
    nc = _NC_CACHE[key]
    in_maps = host_pack(inputs)
    res = run_bass_kernel_spmd(nc, in_maps, list(range(N_CORES))).results
    out = [np.asarray(res[c]["o"], dtype=np.float32) for c in range(N_CORES)]
    pi = np.concatenate([o[:, :DPI] for o in out], axis=0)
    vf = np.concatenate([o[:, DPI:] for o in out], axis=0)
    return pi, vf
